# revision 24
# baseline (speedup 1.0000x reference)
"""MultiHeadSemGConv Trainium2 kernel.

Computes, for x:[B,N,CIN], W:[H,2,CIN,HC], e:[H,N*K], bias:[H,HC],
rows/cols:[N*K] (int32 edge list):

    h = einsum('bnc,hscd->shbnd', x, W)             # two projections per head
    A = softmax(scatter(e at (rows,cols), NEG))     # [H,N,N]
    out[h,b] = diag(A)*h0 + (A - diag)@h1 + bias    # -> [B,N,H*HC]

Strategy: pure data-parallel over batch across 8 NeuronCores.  The tiny
[H,98,98] adjacency softmax is precomputed on host; the heavy lifting
(x projection + graph mixing over 100MB of activations) runs on device:

  per core (128 samples, x pre-cast to fp16 host-side):
    - load x TRANSPOSED straight into SBUF chunk tiles
      [c(2x128), 16*98+32 cols] via the DMA XBAR transpose
      (fp16, 2KB-ish ring efficiency, no PE/engine transpose work);
      the 32-col chunk overlap keeps every per-sample phase-1
      stationary at m=128
    - phase 1, per sample b: h[128,512] = xT[:, 98b:98b+128].T @ Wall
      (2 accumulating fp16 matmuls, f32 PSUM), 2 samples per PSUM tile;
      rows [:98] copied into one of two persistent h tiles whose row 98
      holds the bias pattern
    - phase 2, per 8-sample group, per head: ONE matmul with the
      host-built A_off^T (contract k=99: 98 nodes + bias row), then a
      fused DVE op  out = dg (.) h0 + psum  adds the diagonal part.
      Phase 2 of group g is interleaved into phase 1 of group g+1.
    - DMA out f32
"""

import os
import sys

import numpy as np

try:
    import concourse.bass as bass  # noqa: F401
except Exception:  # pragma: no cover - fresh grading dir fallback
    for p in ("/opt/trn_rl_repo", "/root/.axon_site/_ro/trn_rl_repo"):
        if os.path.isdir(p) and p not in sys.path:
            sys.path.insert(0, p)
    import concourse.bass as bass  # noqa: F401

# ---------------------------------------------------------------- constants
NLM = 98          # landmarks (graph nodes)
HEADS = 4
CIN = 256
HC = 64
HD = 512          # h width = 2 (s) * 4 (heads) * 64 (d)
B = 1024
NCORES = 8
NS = B // NCORES  # samples per core = 128
P = 128
G = 8             # samples per output group
NGRP = NS // G    # 16 groups per core
NEG = -9e15

CHS = 16                    # samples per xT chunk
NCH = NS // CHS             # 8 chunks
CHW = CHS * NLM             # 1568 cols per chunk
OVL = 32                    # overlap so every phase-1 stationary is m=128
CHR = CHW + OVL             # 1600 loaded cols per chunk
XROWS = NS * NLM + OVL      # 12576 padded x rows per core

_CACHE = {}


def _build_nc():
    import concourse.mybir as mybir
    import concourse.tile as tile
    from concourse import bacc

    f16 = mybir.dt.float16
    f32 = mybir.dt.float32
    MUL = mybir.AluOpType.mult
    ADD = mybir.AluOpType.add

    nc = bacc.Bacc(None, target_bir_lowering=False)

    x16 = nc.dram_tensor("x16", [XROWS, CIN], f16, kind="ExternalInput")
    wall = nc.dram_tensor("wall", [P, 2, HD], f16, kind="ExternalInput")
    gmat = nc.dram_tensor("gmat", [P, HEADS * P], f16, kind="ExternalInput")
    dgvt = nc.dram_tensor("dgvt", [NLM, HEADS], f32, kind="ExternalInput")
    brow = nc.dram_tensor("brow", [1, G * HD], f16, kind="ExternalInput")
    out = nc.dram_tensor("out", [NS * NLM, CIN], f32, kind="ExternalOutput")

    with tile.TileContext(nc) as tc:
        with (
            tc.tile_pool(name="const", bufs=1) as constp,
            tc.tile_pool(name="xt", bufs=1) as xtp,
            tc.tile_pool(name="hg", bufs=1) as hgp,
            tc.tile_pool(name="osb", bufs=4) as osbp,
            tc.tile_pool(name="phs", bufs=2, space="PSUM") as phsp,
            tc.tile_pool(name="pout", bufs=4, space="PSUM") as poutp,
        ):
            xt = [
                xtp.tile([P, 2, CHR], f16, tag=f"xt{k}", name=f"xt{k}")
                for k in range(NCH)
            ]
            wall_sb = constp.tile([P, 2, HD], f16, tag="wall")
            gm_sb = constp.tile([P, HEADS * P], f16, tag="gmat")
            dgv_sb = constp.tile([NLM, HEADS], f32, tag="dgv")
            hgt = [
                hgp.tile([P, G * HD], f16, tag=f"hg{i}", name=f"hg{i}")
                for i in range(2)
            ]

            # head: chunk 0 in row-quarters on both HWDGE queues, consts
            # interleaved so the first phase-1 sample is ready ~2.5us in
            nc.scalar.dma_start(wall_sb[:], wall[:])
            for q in range(4):
                r0 = q * 400
                nc.sync.dma_start(
                    xt[0][:, 0, r0 : r0 + 400],
                    x16[r0 : r0 + 400, 0:128],
                    transpose=True,
                )
                nc.scalar.dma_start(
                    xt[0][:, 1, r0 : r0 + 400],
                    x16[r0 : r0 + 400, 128:256],
                    transpose=True,
                )
            nc.sync.dma_start(gm_sb[:], gmat[:])
            nc.scalar.dma_start(dgv_sb[:], dgvt[:])
            nc.sync.dma_start(hgt[0][98:99, :], brow[:])
            nc.sync.dma_start(hgt[1][98:99, :], brow[:])

            # The sync queue carries ONLY chunk transposes (paced below);
            # store descriptor-gen lives on the otherwise-idle gpsimd
            # queue.  Keeping them apart matters: every queue is in-order,
            # and a store gen's osb data-dependency would head-of-line
            # block later chunk loads (and vice versa).
            def emit_chunk_dma(k):
                base = k * CHW
                for cc in range(2):
                    nc.sync.dma_start(
                        xt[k][:, cc, :],
                        x16[base : base + CHR, cc * P : (cc + 1) * P],
                        transpose=True,
                    )

            hg3s = [h[:].rearrange("p (s f) -> p s f", s=G) for h in hgt]

            def emit_p2_head(gi, hd, osb3):
                """Phase 2 for one head of group gi: one k=99 matmul
                (A_off^T + bias row), then fused  out = dg (.) h0 + psum."""
                hg3 = hg3s[gi % 2]
                pouts = poutp.tile([P, G * HC], f32, tag="pout")
                po3 = pouts[:].rearrange("p (s f) -> p s f", s=G)
                nc.tensor.matmul(
                    po3,
                    gm_sb[0:99, hd * P : (hd + 1) * P],
                    hg3[0:99, :, 256 + hd * HC : 256 + (hd + 1) * HC],
                    start=True,
                    stop=True,
                )
                nc.vector.scalar_tensor_tensor(
                    out=osb3[:, :, hd * HC : (hd + 1) * HC],
                    in0=hg3[0:98, :, hd * HC : (hd + 1) * HC],
                    scalar=dgv_sb[:, hd : hd + 1],
                    in1=po3[0:98],
                    op0=MUL,
                    op1=ADD,
                )

            def store(gi, osb3, s0=0, s1=G):
                ov = out[gi * G * NLM : (gi + 1) * G * NLM, :].rearrange(
                    "(s i) c -> i s c", s=G
                )
                nc.gpsimd.dma_start(ov[:, s0:s1], osb3[:, s0:s1])

            osb_t = {}

            def emit_b_phase1(gi, prev):
                """Phase 1 for G samples of gi; phase 2 of group `prev`
                interleaved between the pairs."""
                hgrp = hgt[gi % 2]
                if prev is not None:
                    osb = osbp.tile([NLM, G * 256], f32, tag="osb")
                    osb3 = osb[:].rearrange("p (s c) -> p s c", s=G)
                    osb_t[prev] = osb3
                ck = gi // 2
                for pi in range(G // 2):
                    hps = phsp.tile([P, 2, HD], f32, tag="hps")
                    for a in range(2):
                        b = gi * G + pi * 2 + a
                        lb = b - ck * CHS
                        for cc in range(2):
                            nc.tensor.matmul(
                                hps[:, a, :],
                                xt[ck][:, cc, NLM * lb : NLM * lb + P],
                                wall_sb[:, cc, :],
                                start=(cc == 0),
                                stop=(cc == 1),
                            )
                    dst = hgrp[0:98, pi * 2 * HD : (pi + 1) * 2 * HD].rearrange(
                        "p (a f) -> p a f", a=2
                    )
                    if pi == 0:
                        nc.vector.tensor_copy(dst, hps[0:98])
                    else:
                        nc.scalar.copy(out=dst, in_=hps[0:98])
                    if prev is not None:
                        if pi < 3:
                            emit_p2_head(prev, pi, osb_t[prev])
                        else:
                            emit_p2_head(prev, 3, osb_t[prev])
                            store(prev, osb_t[prev])

            def emit_p2_flush(gi):
                """Phase 2 for the final group, split in half-groups with
                split stores for a short kernel tail."""
                osb = osbp.tile([NLM, G * 256], f32, tag="osb")
                osb3 = osb[:].rearrange("p (s c) -> p s c", s=G)
                hg3 = hg3s[gi % 2]
                for half in range(2):
                    s0, s1 = half * 4, half * 4 + 4
                    for hd in range(HEADS):
                        pouts = poutp.tile([P, G * HC], f32, tag="pout")
                        po3 = pouts[:].rearrange("p (s f) -> p s f", s=G)
                        nc.tensor.matmul(
                            po3[:, s0:s1, :],
                            gm_sb[0:99, hd * P : (hd + 1) * P],
                            hg3[0:99, s0:s1, 256 + hd * HC : 256 + (hd + 1) * HC],
                            start=True,
                            stop=True,
                        )
                        nc.vector.scalar_tensor_tensor(
                            out=osb3[:, s0:s1, hd * HC : (hd + 1) * HC],
                            in0=hg3[0:98, s0:s1, hd * HC : (hd + 1) * HC],
                            scalar=dgv_sb[:, hd : hd + 1],
                            in1=po3[0:98, s0:s1],
                            op0=MUL,
                            op1=ADD,
                        )
                    if half == 0:
                        store(gi, osb3, 0, 4)
                    else:
                        store(gi, osb3, 4, 6)
                        store(gi, osb3, 6, 8)

            # ---- main emission ------------------------------------------
            ch_emitted = 1
            prev = None
            for gi in range(NGRP):
                while ch_emitted <= min(gi // 2 + 1, NCH - 1):
                    emit_chunk_dma(ch_emitted)
                    ch_emitted += 1
                emit_b_phase1(gi, prev)
                prev = gi
            emit_p2_flush(prev)

    nc.compile()
    return nc


def _host_prep(W, e, bias, rows, cols):
    """Precompute fp16 device constants from the small parameter tensors."""
    W = np.asarray(W, np.float32)
    e = np.asarray(e, np.float32)
    bias = np.asarray(bias, np.float32)
    rows = np.asarray(rows, np.int64)
    cols = np.asarray(cols, np.int64)

    logits = np.full((HEADS, NLM, NLM), NEG, np.float64)
    logits[:, rows, cols] = e.astype(np.float64)
    m = logits.max(axis=-1, keepdims=True)
    p = np.exp(logits - m)
    A = p / p.sum(axis=-1, keepdims=True)            # [H, N, N]
    dg = np.einsum("hii->hi", A).copy()              # [H, N]
    A_off = A.copy()
    np.einsum("hii->hi", A_off)[:] = 0.0

    # Wall: [c, (s, h, d)] -> chunked [128, 2, 512]
    wr = W.transpose(2, 1, 0, 3).reshape(CIN, 2 * HEADS * HC)   # [c, shd]
    wall = np.ascontiguousarray(
        wr.reshape(2, P, 2 * HEADS * HC).transpose(1, 0, 2)
    ).astype(np.float16)

    # graph matrices: [j, (head, i)]; row 98 = all-ones bias row
    gm = np.zeros((P, HEADS, P), np.float32)
    for h in range(HEADS):
        gm[:NLM, h, :NLM] = A_off[h].T
        gm[NLM, h, :NLM] = 1.0
    gmat = np.ascontiguousarray(gm.reshape(P, HEADS * P)).astype(np.float16)

    dgvt = np.ascontiguousarray(dg.T).astype(np.float32)        # [98, 4]

    # bias row pattern for hgrp row 98: [s, (part, h, d)], part-1 = bias
    br = np.zeros((G, 2, HEADS * HC), np.float32)
    br[:, 1, :] = bias.reshape(HEADS * HC)
    brow = np.ascontiguousarray(br.reshape(1, G * HD)).astype(np.float16)

    return {"wall": wall, "gmat": gmat, "dgvt": dgvt, "brow": brow}


def kernel(x, W, e, bias, rows, cols):
    from concourse.bass_utils import run_bass_kernel_spmd

    if "nc" not in _CACHE:
        _CACHE["nc"] = _build_nc()
    nc = _CACHE["nc"]

    consts = _host_prep(W, e, bias, rows, cols)
    x16 = np.asarray(x, np.float32).reshape(B * NLM, CIN).astype(np.float16)

    in_maps = []
    for ci in range(NCORES):
        shard = np.zeros((XROWS, CIN), np.float16)
        shard[: NS * NLM] = x16[ci * NS * NLM : (ci + 1) * NS * NLM]
        in_maps.append({"x16": shard, **consts})

    res = run_bass_kernel_spmd(
        nc,
        in_maps,
        core_ids=list(range(NCORES)),
        trace=bool(int(os.environ.get("KERNEL_TRACE", "0"))),
    )
    _CACHE["last_results"] = res

    out = np.concatenate(
        [r["out"].reshape(NS, NLM, HEADS * HC) for r in res.results], axis=0
    )
    return out


# revision 26
# speedup vs baseline: 1.1358x; 1.1358x over previous
"""MultiHeadSemGConv Trainium2 kernel.

Computes, for x:[B,N,CIN], W:[H,2,CIN,HC], e:[H,N*K], bias:[H,HC],
rows/cols:[N*K] (int32 edge list):

    h = einsum('bnc,hscd->shbnd', x, W)             # two projections per head
    A = softmax(scatter(e at (rows,cols), NEG))     # [H,N,N]
    out[h,b] = diag(A)*h0 + (A - diag)@h1 + bias    # -> [B,N,H*HC]

Strategy: pure data-parallel over batch across 8 NeuronCores.  The tiny
[H,98,98] adjacency softmax is precomputed on host; the heavy lifting
(x projection + graph mixing over 100MB of activations) runs on device:

  per core (128 samples, x pre-cast to fp16 host-side):
    - load x TRANSPOSED straight into SBUF chunk tiles
      [c(2x128), 16*98+32 cols] via the DMA XBAR transpose
      (fp16, 2KB-ish ring efficiency, no PE/engine transpose work);
      the 32-col chunk overlap keeps every per-sample phase-1
      stationary at m=128
    - phase 1, per sample b: h[128,512] = xT[:, 98b:98b+128].T @ Wall
      (2 accumulating fp16 matmuls, f32 PSUM), 2 samples per PSUM tile;
      rows [:98] copied into one of two persistent h tiles whose row 98
      holds the bias pattern
    - phase 2, per 8-sample group, per head: ONE matmul with the
      host-built A_off^T (contract k=99: 98 nodes + bias row), then a
      fused DVE op  out = dg (.) h0 + psum  adds the diagonal part.
      Phase 2 of group g is interleaved into phase 1 of group g+1.
    - DMA out f32
"""

import os
import sys

import numpy as np

try:
    import concourse.bass as bass  # noqa: F401
except Exception:  # pragma: no cover - fresh grading dir fallback
    for p in ("/opt/trn_rl_repo", "/root/.axon_site/_ro/trn_rl_repo"):
        if os.path.isdir(p) and p not in sys.path:
            sys.path.insert(0, p)
    import concourse.bass as bass  # noqa: F401

# ---------------------------------------------------------------- constants
NLM = 98          # landmarks (graph nodes)
HEADS = 4
CIN = 256
HC = 64
HD = 512          # h width = 2 (s) * 4 (heads) * 64 (d)
B = 1024
NCORES = 8
NS = B // NCORES  # samples per core = 128
P = 128
G = 8             # samples per output group
NGRP = NS // G    # 16 groups per core
NEG = -9e15

CHS = 16                    # samples per xT chunk
NCH = NS // CHS             # 8 chunks
CHW = CHS * NLM             # 1568 cols per chunk
OVL = 32                    # overlap so every phase-1 stationary is m=128
CHR = CHW + OVL             # 1600 loaded cols per chunk
XROWS = NS * NLM + OVL      # 12576 padded x rows per core

_CACHE = {}


def _build_nc():
    import concourse.mybir as mybir
    import concourse.tile as tile
    from concourse import bacc

    f16 = mybir.dt.float16
    f32 = mybir.dt.float32
    MUL = mybir.AluOpType.mult
    ADD = mybir.AluOpType.add

    nc = bacc.Bacc(None, target_bir_lowering=False)

    x16 = nc.dram_tensor("x16", [XROWS, CIN], f16, kind="ExternalInput")
    wall = nc.dram_tensor("wall", [P, 2, HD], f16, kind="ExternalInput")
    gmat = nc.dram_tensor("gmat", [P, HEADS * P], f16, kind="ExternalInput")
    dgvt = nc.dram_tensor("dgvt", [NLM, HEADS], f32, kind="ExternalInput")
    brow = nc.dram_tensor("brow", [1, G * HD], f16, kind="ExternalInput")
    out = nc.dram_tensor("out", [NS * NLM, CIN], f32, kind="ExternalOutput")

    with tile.TileContext(nc) as tc:
        with (
            tc.tile_pool(name="const", bufs=1) as constp,
            tc.tile_pool(name="xt", bufs=1) as xtp,
            tc.tile_pool(name="hg", bufs=1) as hgp,
            tc.tile_pool(name="osb", bufs=4) as osbp,
            tc.tile_pool(name="phs", bufs=2, space="PSUM") as phsp,
            tc.tile_pool(name="pout", bufs=4, space="PSUM") as poutp,
        ):
            xt = [
                xtp.tile([P, 2, CHR], f16, tag=f"xt{k}", name=f"xt{k}")
                for k in range(NCH)
            ]
            wall_sb = constp.tile([P, 2, HD], f16, tag="wall")
            gm_sb = constp.tile([P, HEADS * P], f16, tag="gmat")
            dgv_sb = constp.tile([NLM, HEADS], f32, tag="dgv")
            hgt = [
                hgp.tile([P, G * HD], f16, tag=f"hg{i}", name=f"hg{i}")
                for i in range(2)
            ]

            # head: chunk 0 in row-quarters on both HWDGE queues, consts
            # interleaved so the first phase-1 sample is ready ~2.5us in
            nc.scalar.dma_start(wall_sb[:], wall[:])
            for q in range(4):
                r0 = q * 400
                nc.sync.dma_start(
                    xt[0][:, 0, r0 : r0 + 400],
                    x16[r0 : r0 + 400, 0:128],
                    transpose=True,
                )
                nc.scalar.dma_start(
                    xt[0][:, 1, r0 : r0 + 400],
                    x16[r0 : r0 + 400, 128:256],
                    transpose=True,
                )
            nc.sync.dma_start(gm_sb[:], gmat[:])
            nc.scalar.dma_start(dgv_sb[:], dgvt[:])
            nc.sync.dma_start(hgt[0][98:99, :], brow[:])
            nc.sync.dma_start(hgt[1][98:99, :], brow[:])

            # The sync queue carries ONLY chunk transposes (paced below);
            # store descriptor-gen lives on the otherwise-idle gpsimd
            # queue.  Keeping them apart matters: every queue is in-order,
            # and a store gen's osb data-dependency would head-of-line
            # block later chunk loads (and vice versa).
            def emit_chunk_dma(k):
                base = k * CHW
                for cc in range(2):
                    nc.sync.dma_start(
                        xt[k][:, cc, :],
                        x16[base : base + CHR, cc * P : (cc + 1) * P],
                        transpose=True,
                    )

            hg3s = [h[:].rearrange("p (s f) -> p s f", s=G) for h in hgt]

            def emit_p2_head(gi, hd, osb3):
                """Phase 2 for one head of group gi: one k=99 matmul
                (A_off^T + bias row), then fused  out = dg (.) h0 + psum."""
                hg3 = hg3s[gi % 2]
                pouts = poutp.tile([P, G * HC], f32, tag="pout")
                po3 = pouts[:].rearrange("p (s f) -> p s f", s=G)
                nc.tensor.matmul(
                    po3,
                    gm_sb[0:99, hd * P : (hd + 1) * P],
                    hg3[0:99, :, 256 + hd * HC : 256 + (hd + 1) * HC],
                    start=True,
                    stop=True,
                )
                nc.vector.scalar_tensor_tensor(
                    out=osb3[:, :, hd * HC : (hd + 1) * HC],
                    in0=hg3[0:98, :, hd * HC : (hd + 1) * HC],
                    scalar=dgv_sb[:, hd : hd + 1],
                    in1=po3[0:98],
                    op0=MUL,
                    op1=ADD,
                )

            def store(gi, osb3, s0=0, s1=G):
                ov = out[gi * G * NLM : (gi + 1) * G * NLM, :].rearrange(
                    "(s i) c -> i s c", s=G
                )
                nc.sync.dma_start(ov[:, s0:s1], osb3[:, s0:s1])

            osb_t = {}

            def emit_b_phase1(gi, prev):
                """Phase 1 for G samples of gi; phase 2 of group `prev`
                interleaved between the pairs."""
                hgrp = hgt[gi % 2]
                if prev is not None:
                    osb = osbp.tile([NLM, G * 256], f32, tag="osb")
                    osb3 = osb[:].rearrange("p (s c) -> p s c", s=G)
                    osb_t[prev] = osb3
                ck = gi // 2
                for pi in range(G // 2):
                    hps = phsp.tile([P, 2, HD], f32, tag="hps")
                    for a in range(2):
                        b = gi * G + pi * 2 + a
                        lb = b - ck * CHS
                        for cc in range(2):
                            nc.tensor.matmul(
                                hps[:, a, :],
                                xt[ck][:, cc, NLM * lb : NLM * lb + P],
                                wall_sb[:, cc, :],
                                start=(cc == 0),
                                stop=(cc == 1),
                            )
                    dst = hgrp[0:98, pi * 2 * HD : (pi + 1) * 2 * HD].rearrange(
                        "p (a f) -> p a f", a=2
                    )
                    if pi == 0:
                        nc.vector.tensor_copy(dst, hps[0:98])
                    else:
                        nc.scalar.copy(out=dst, in_=hps[0:98])
                    if prev is not None:
                        if pi < 3:
                            emit_p2_head(prev, pi, osb_t[prev])
                        else:
                            emit_p2_head(prev, 3, osb_t[prev])
                            store(prev, osb_t[prev])

            def emit_p2_flush(gi):
                """Phase 2 for the final group, split in half-groups with
                split stores for a short kernel tail."""
                osb = osbp.tile([NLM, G * 256], f32, tag="osb")
                osb3 = osb[:].rearrange("p (s c) -> p s c", s=G)
                hg3 = hg3s[gi % 2]
                for half in range(2):
                    s0, s1 = half * 4, half * 4 + 4
                    for hd in range(HEADS):
                        pouts = poutp.tile([P, G * HC], f32, tag="pout")
                        po3 = pouts[:].rearrange("p (s f) -> p s f", s=G)
                        nc.tensor.matmul(
                            po3[:, s0:s1, :],
                            gm_sb[0:99, hd * P : (hd + 1) * P],
                            hg3[0:99, s0:s1, 256 + hd * HC : 256 + (hd + 1) * HC],
                            start=True,
                            stop=True,
                        )
                        nc.vector.scalar_tensor_tensor(
                            out=osb3[:, s0:s1, hd * HC : (hd + 1) * HC],
                            in0=hg3[0:98, s0:s1, hd * HC : (hd + 1) * HC],
                            scalar=dgv_sb[:, hd : hd + 1],
                            in1=po3[0:98, s0:s1],
                            op0=MUL,
                            op1=ADD,
                        )
                    if half == 0:
                        store(gi, osb3, 0, 4)
                    else:
                        store(gi, osb3, 4, 6)
                        store(gi, osb3, 6, 8)

            # ---- main emission ------------------------------------------
            ch_emitted = 1
            prev = None
            for gi in range(NGRP):
                while ch_emitted <= min(gi // 2 + 2, NCH - 1):
                    emit_chunk_dma(ch_emitted)
                    ch_emitted += 1
                emit_b_phase1(gi, prev)
                prev = gi
            emit_p2_flush(prev)

    nc.compile()
    return nc


def _host_prep(W, e, bias, rows, cols):
    """Precompute fp16 device constants from the small parameter tensors."""
    W = np.asarray(W, np.float32)
    e = np.asarray(e, np.float32)
    bias = np.asarray(bias, np.float32)
    rows = np.asarray(rows, np.int64)
    cols = np.asarray(cols, np.int64)

    logits = np.full((HEADS, NLM, NLM), NEG, np.float64)
    logits[:, rows, cols] = e.astype(np.float64)
    m = logits.max(axis=-1, keepdims=True)
    p = np.exp(logits - m)
    A = p / p.sum(axis=-1, keepdims=True)            # [H, N, N]
    dg = np.einsum("hii->hi", A).copy()              # [H, N]
    A_off = A.copy()
    np.einsum("hii->hi", A_off)[:] = 0.0

    # Wall: [c, (s, h, d)] -> chunked [128, 2, 512]
    wr = W.transpose(2, 1, 0, 3).reshape(CIN, 2 * HEADS * HC)   # [c, shd]
    wall = np.ascontiguousarray(
        wr.reshape(2, P, 2 * HEADS * HC).transpose(1, 0, 2)
    ).astype(np.float16)

    # graph matrices: [j, (head, i)]; row 98 = all-ones bias row
    gm = np.zeros((P, HEADS, P), np.float32)
    for h in range(HEADS):
        gm[:NLM, h, :NLM] = A_off[h].T
        gm[NLM, h, :NLM] = 1.0
    gmat = np.ascontiguousarray(gm.reshape(P, HEADS * P)).astype(np.float16)

    dgvt = np.ascontiguousarray(dg.T).astype(np.float32)        # [98, 4]

    # bias row pattern for hgrp row 98: [s, (part, h, d)], part-1 = bias
    br = np.zeros((G, 2, HEADS * HC), np.float32)
    br[:, 1, :] = bias.reshape(HEADS * HC)
    brow = np.ascontiguousarray(br.reshape(1, G * HD)).astype(np.float16)

    return {"wall": wall, "gmat": gmat, "dgvt": dgvt, "brow": brow}


def kernel(x, W, e, bias, rows, cols):
    from concourse.bass_utils import run_bass_kernel_spmd

    if "nc" not in _CACHE:
        _CACHE["nc"] = _build_nc()
    nc = _CACHE["nc"]

    consts = _host_prep(W, e, bias, rows, cols)
    x16 = np.asarray(x, np.float32).reshape(B * NLM, CIN).astype(np.float16)

    in_maps = []
    for ci in range(NCORES):
        shard = np.zeros((XROWS, CIN), np.float16)
        shard[: NS * NLM] = x16[ci * NS * NLM : (ci + 1) * NS * NLM]
        in_maps.append({"x16": shard, **consts})

    res = run_bass_kernel_spmd(
        nc,
        in_maps,
        core_ids=list(range(NCORES)),
        trace=bool(int(os.environ.get("KERNEL_TRACE", "0"))),
    )
    _CACHE["last_results"] = res

    out = np.concatenate(
        [r["out"].reshape(NS, NLM, HEADS * HC) for r in res.results], axis=0
    )
    return out
